# revision 3
# baseline (speedup 1.0000x reference)
"""GNN message-passing layer on 8 TRN2 NeuronCores.

Computes out = relu((adj^T @ x / deg) @ U^T) for N=8192 nodes, D=512 dims.

Sharding: columns of adj (= output rows) are split across the 8 cores;
x and U are replicated, so each core computes a [1024, 512] output slab
with no collectives.

Per-core kernel (all matmuls in bf16, accumulating in f32 PSUM):
  aggT[d, i] = sum_j x[j, d] * A[j, i]   via x-chunk weights, A streamed
  deg[i]     = sum_j A[j, i]             via an all-ones weight matrix
                                         (result replicated on 128 partitions)
  agg_scaled = aggT * (1/deg)  (free-dim broadcast multiply during PSUM evac)
  out[i, k]  = relu(sum_d agg_scaled[d, i] * U^T[d, k])
"""

import sys

if "/opt/trn_rl_repo" not in sys.path:
    sys.path.insert(0, "/opt/trn_rl_repo")

import numpy as np

import concourse.bacc as bacc
import concourse.mybir as mybir
import concourse.tile as tile
from concourse.bass_utils import run_bass_kernel_spmd

N = 8192          # nodes
D = 512           # node dim
NCORES = 8
SH = N // NCORES  # 1024 adj columns (output rows) per core
NJ = N // 128     # 64 contraction tiles
XG = 8            # j-tiles per x load group
NG = NJ // XG     # 8 x groups
F32 = mybir.dt.float32
BF16 = mybir.dt.bfloat16
I32 = mybir.dt.int32

_compiled = None


def _build():
    nc = bacc.Bacc("TRN2", target_bir_lowering=False, debug=False, num_devices=NCORES)
    x_d = nc.dram_tensor("x", [N, D], F32, kind="ExternalInput").ap()
    adj_d = nc.dram_tensor("adj", [N, SH], I32, kind="ExternalInput").ap()
    ut_d = nc.dram_tensor("ut", [D, D], F32, kind="ExternalInput").ap()
    out_d = nc.dram_tensor("out", [SH, D], F32, kind="ExternalOutput").ap()

    x_view = x_d.rearrange("(t p) d -> p t d", p=128)   # [128, 64, 512]
    ut_view = ut_d.rearrange("(c p) k -> p c k", p=128)  # [128, 4, 512]

    with tile.TileContext(nc) as tc:
        with (
            tc.tile_pool(name="xw", bufs=1) as xw_pool,
            tc.tile_pool(name="xs", bufs=2) as xs_pool,
            tc.tile_pool(name="ust", bufs=1) as ust_pool,
            tc.tile_pool(name="aint", bufs=6) as aint_pool,
            tc.tile_pool(name="abf", bufs=6) as abf_pool,
            tc.tile_pool(name="cons", bufs=1) as cons_pool,
            tc.tile_pool(name="evac", bufs=2) as evac_pool,
            tc.tile_pool(name="osb", bufs=3) as osb_pool,
            tc.tile_pool(name="pacc", bufs=1, space="PSUM") as pacc_pool,
            tc.tile_pool(name="pout", bufs=2, space="PSUM") as pout_pool,
        ):
            xg_tiles = [None] * NG

            def load_x_group(g):
                xs = xs_pool.tile([128, XG, D], F32, tag="xs")
                nc.sync.dma_start(xs[:], x_view[:, g * XG:(g + 1) * XG, :])
                xg = xw_pool.tile([128, XG, D], BF16, tag=f"xg{g}")
                nc.scalar.copy(xg[:], xs[:])
                xg_tiles[g] = xg

            load_x_group(0)
            load_x_group(1)

            ones = cons_pool.tile([128, 128], BF16)
            nc.vector.memset(ones[:], 1.0)
            u_st = ust_pool.tile([128, 4, D], F32)
            nc.sync.dma_start(u_st[:], ut_view[:])
            u_bf = cons_pool.tile([128, 4, D], BF16)
            nc.scalar.copy(u_bf[:], u_st[:])

            for h in range(2):
                agg_ps = [
                    pacc_pool.tile([128, D], F32, tag=f"agg{c}", name=f"agg{c}")
                    for c in range(4)
                ]
                deg_ps = pacc_pool.tile([128, D], F32, tag="deg")
                for j in range(NJ):
                    # prefetch x groups two groups ahead during the first half
                    if h == 0 and j % XG == 0 and j // XG + 2 < NG:
                        load_x_group(j // XG + 2)
                    a_int = aint_pool.tile([128, D], I32, tag="aint")
                    nc.sync.dma_start(
                        a_int[:], adj_d[j * 128:(j + 1) * 128, h * D:(h + 1) * D]
                    )
                    a_bf = abf_pool.tile([128, D], BF16, tag="abf")
                    nc.vector.tensor_copy(a_bf[:], a_int[:])
                    st, sp = j == 0, j == NJ - 1
                    nc.tensor.matmul(deg_ps[:], ones[:], a_bf[:], start=st, stop=sp)
                    xg = xg_tiles[j // XG]
                    for c in range(4):
                        nc.tensor.matmul(
                            agg_ps[c][:],
                            xg[:, j % XG, c * 128:(c + 1) * 128],
                            a_bf[:],
                            start=st,
                            stop=sp,
                        )

                recip = evac_pool.tile([128, D], F32, tag="recip")
                nc.vector.reciprocal(recip[:], deg_ps[:])
                agg_sc = [
                    evac_pool.tile([128, D], BF16, tag=f"aggsc{c}", name=f"aggsc{c}")
                    for c in range(4)
                ]
                for c in range(4):
                    nc.vector.tensor_mul(agg_sc[c][:], agg_ps[c][:], recip[:])

                for ic in range(4):
                    out_ps = pout_pool.tile([128, D], F32, tag="outps")
                    for c in range(4):
                        nc.tensor.matmul(
                            out_ps[:],
                            agg_sc[c][:, ic * 128:(ic + 1) * 128],
                            u_bf[:, c, :],
                            start=c == 0,
                            stop=c == 3,
                        )
                    out_sb = osb_pool.tile([128, D], F32, tag="osb")
                    nc.scalar.activation(
                        out_sb[:], out_ps[:], mybir.ActivationFunctionType.Relu
                    )
                    r0 = (h * 4 + ic) * 128
                    nc.sync.dma_start(out_d[r0:r0 + 128, :], out_sb[:])

    nc.compile()
    return nc


def _get_compiled():
    global _compiled
    if _compiled is None:
        _compiled = _build()
    return _compiled


def _run(x, adj, u, **spmd_kwargs):
    nc = _get_compiled()
    x = np.ascontiguousarray(np.asarray(x, dtype=np.float32))
    adj = np.asarray(adj, dtype=np.int32)
    ut = np.ascontiguousarray(np.asarray(u, dtype=np.float32).T)
    in_maps = [
        {
            "x": x,
            "ut": ut,
            "adj": np.ascontiguousarray(adj[:, c * SH:(c + 1) * SH]),
        }
        for c in range(NCORES)
    ]
    res = run_bass_kernel_spmd(nc, in_maps, core_ids=list(range(NCORES)), **spmd_kwargs)
    out = np.concatenate([res.results[c]["out"] for c in range(NCORES)], axis=0)
    return out, res


def kernel(x, adj, U):
    out, _ = _run(x, adj, U)
    return out


# revision 4
# speedup vs baseline: 1.0458x; 1.0458x over previous
"""GNN message-passing layer on 8 TRN2 NeuronCores.

Computes out = relu((adj^T @ x / deg) @ U^T) for N=8192 nodes, D=512 dims.

Sharding: columns of adj (= output rows) are split across the 8 cores;
x and U are replicated, so each core computes a [1024, 512] output slab
with no collectives.

Host-side restaging (pure layout shuffles, no arithmetic): every DRAM
tensor is laid out partition-major so each SBUF partition reads one long
contiguous run (16-32KB) — small per-row DMA packets were the original
bottleneck. The int32->bf16 and f32->bf16 casts ride the SWDGE DMA
engines for free.

Per-core kernel (all matmuls in bf16, accumulating in f32 PSUM):
  aggT[d, i] = sum_j x[j, d] * A[j, i]   via x-chunk weights, A streamed
  deg[i]     = sum_j A[j, i]             via an all-ones weight matrix
                                         (result replicated on 128 partitions)
  agg_scaled = aggT * (1/deg)  (free-dim broadcast multiply during PSUM evac)
  out[i, k]  = relu(sum_d agg_scaled[d, i] * U^T[d, k])
"""

import sys

if "/opt/trn_rl_repo" not in sys.path:
    sys.path.insert(0, "/opt/trn_rl_repo")

import numpy as np

import concourse.bacc as bacc
import concourse.mybir as mybir
import concourse.tile as tile
from concourse.bass_utils import run_bass_kernel_spmd

N = 8192          # nodes
D = 512           # node dim
NCORES = 8
SH = N // NCORES  # 1024 adj columns (output rows) per core
NJ = N // 128     # 64 contraction tiles
XG = 8            # j-tiles per load group
NG = NJ // XG     # 8 groups
F32 = mybir.dt.float32
BF16 = mybir.dt.bfloat16
I32 = mybir.dt.int32

_compiled = None


def _build():
    nc = bacc.Bacc("TRN2", target_bir_lowering=False, debug=False, num_devices=NCORES)
    # partition-major layouts (see _run for the host-side shuffles)
    x_d = nc.dram_tensor("x", [128, NJ, D], F32, kind="ExternalInput").ap()
    adj_d = nc.dram_tensor("adj", [2, 128, NJ, D], I32, kind="ExternalInput").ap()
    ut_d = nc.dram_tensor("ut", [128, 4, D], F32, kind="ExternalInput").ap()
    out_d = nc.dram_tensor("out", [128, 8, D], F32, kind="ExternalOutput").ap()

    with tile.TileContext(nc) as tc:
        with (
            tc.tile_pool(name="xw", bufs=1) as xw_pool,
            tc.tile_pool(name="abf", bufs=3) as abf_pool,
            tc.tile_pool(name="cons", bufs=1) as cons_pool,
            tc.tile_pool(name="evac", bufs=2) as evac_pool,
            tc.tile_pool(name="osb", bufs=2) as osb_pool,
            tc.tile_pool(name="pacc", bufs=1, space="PSUM") as pacc_pool,
            tc.tile_pool(name="pout", bufs=2, space="PSUM") as pout_pool,
        ):
            ones = cons_pool.tile([128, 128], BF16)
            nc.vector.memset(ones[:], 1.0)
            u_bf = cons_pool.tile([128, 4, D], BF16)
            nc.gpsimd.dma_start(u_bf[:], ut_d[:])

            xg_tiles = []
            for g in range(NG):
                xg = xw_pool.tile([128, XG, D], BF16, tag=f"xg{g}", name=f"xg{g}")
                nc.gpsimd.dma_start(xg[:], x_d[:, g * XG:(g + 1) * XG, :])
                xg_tiles.append(xg)

            for h in range(2):
                agg_ps = [
                    pacc_pool.tile([128, D], F32, tag=f"agg{c}", name=f"agg{c}")
                    for c in range(4)
                ]
                deg_ps = pacc_pool.tile([128, D], F32, tag="deg")
                for g in range(NG):
                    a_bf = abf_pool.tile([128, XG, D], BF16, tag="abf")
                    nc.gpsimd.dma_start(
                        a_bf[:], adj_d[h, :, g * XG:(g + 1) * XG, :]
                    )
                    xg = xg_tiles[g]
                    for t in range(XG):
                        j = g * XG + t
                        st, sp = j == 0, j == NJ - 1
                        nc.tensor.matmul(
                            deg_ps[:], ones[:], a_bf[:, t, :], start=st, stop=sp
                        )
                        for c in range(4):
                            nc.tensor.matmul(
                                agg_ps[c][:],
                                xg[:, t, c * 128:(c + 1) * 128],
                                a_bf[:, t, :],
                                start=st,
                                stop=sp,
                            )

                recip = evac_pool.tile([128, D], F32, tag="recip")
                nc.vector.reciprocal(recip[:], deg_ps[:])
                agg_sc = [
                    evac_pool.tile([128, D], BF16, tag=f"aggsc{c}", name=f"aggsc{c}")
                    for c in range(4)
                ]
                for c in range(4):
                    nc.vector.tensor_mul(agg_sc[c][:], agg_ps[c][:], recip[:])

                out_sb = osb_pool.tile([128, 4, D], F32, tag="osb")
                for ic in range(4):
                    out_ps = pout_pool.tile([128, D], F32, tag="outps")
                    for c in range(4):
                        nc.tensor.matmul(
                            out_ps[:],
                            agg_sc[c][:, ic * 128:(ic + 1) * 128],
                            u_bf[:, c, :],
                            start=c == 0,
                            stop=c == 3,
                        )
                    nc.scalar.activation(
                        out_sb[:, ic, :], out_ps[:], mybir.ActivationFunctionType.Relu
                    )
                nc.sync.dma_start(out_d[:, h * 4:(h + 1) * 4, :], out_sb[:])

    nc.compile()
    return nc


def _get_compiled():
    global _compiled
    if _compiled is None:
        _compiled = _build()
    return _compiled


def _run(x, adj, u, **spmd_kwargs):
    nc = _get_compiled()
    x = np.asarray(x, dtype=np.float32)
    adj = np.asarray(adj, dtype=np.int32)
    u = np.asarray(u, dtype=np.float32)

    # x[t*128+p, d] -> x_r[p, t, d]
    x_r = np.ascontiguousarray(x.reshape(NJ, 128, D).transpose(1, 0, 2))
    # U^T[c*128+p, k] -> ut_r[p, c, k]
    ut_r = np.ascontiguousarray(u.T.reshape(4, 128, D).transpose(1, 0, 2))
    in_maps = []
    for core in range(NCORES):
        shard = adj[:, core * SH:(core + 1) * SH]
        # shard[t*128+p, h*512+d] -> adj_r[h, p, t, d]
        adj_r = np.ascontiguousarray(
            shard.reshape(NJ, 128, 2, D).transpose(2, 1, 0, 3)
        )
        in_maps.append({"x": x_r, "ut": ut_r, "adj": adj_r})

    res = run_bass_kernel_spmd(nc, in_maps, core_ids=list(range(NCORES)), **spmd_kwargs)
    # out_r[p, hic, k] -> out[hic*128+p, k], then stack core slabs
    out = np.concatenate(
        [
            res.results[c]["out"].transpose(1, 0, 2).reshape(SH, D)
            for c in range(NCORES)
        ],
        axis=0,
    )
    return out, res


def kernel(x, adj, U):
    out, _ = _run(x, adj, U)
    return out


# revision 6
# speedup vs baseline: 1.1146x; 1.0658x over previous
"""GNN message-passing layer on 8 TRN2 NeuronCores.

Computes out = relu((adj^T @ x / deg) @ U^T) for N=8192 nodes, D=512 dims.

Sharding: columns of adj (= output rows) are split across the 8 cores;
x and U are replicated, so each core computes a [1024, 512] output slab
with no collectives.

Host-side restaging (pure layout shuffles, no arithmetic): every DRAM
tensor is laid out partition-major so each SBUF partition reads one long
contiguous run (16-32KB) — small per-row DMA packets were the original
bottleneck. The int32->bf16 and f32->bf16 casts ride the SWDGE DMA
engines for free.

Per-core kernel (all matmuls in bf16, accumulating in f32 PSUM):
  aggT[d, i] = sum_j x[j, d] * A[j, i]   via x-chunk weights, A streamed
  deg[i]     = sum_j A[j, i]             via an all-ones weight matrix
                                         (result replicated on 128 partitions)
  agg_scaled = aggT * (1/deg)  (free-dim broadcast multiply during PSUM evac)
  out[i, k]  = relu(sum_d agg_scaled[d, i] * U^T[d, k])
"""

import sys

if "/opt/trn_rl_repo" not in sys.path:
    sys.path.insert(0, "/opt/trn_rl_repo")

import numpy as np

import concourse.bacc as bacc
import concourse.mybir as mybir
import concourse.tile as tile
from concourse.bass_utils import run_bass_kernel_spmd

N = 8192          # nodes
D = 512           # node dim
NCORES = 8
SH = N // NCORES  # 1024 adj columns (output rows) per core
NJ = N // 128     # 64 contraction tiles
XG = 4            # j-tiles per load group
NG = NJ // XG     # 16 groups
F32 = mybir.dt.float32
BF16 = mybir.dt.bfloat16
I32 = mybir.dt.int32

_compiled = None


def _build():
    nc = bacc.Bacc("TRN2", target_bir_lowering=False, debug=False, num_devices=NCORES)
    # partition-major layouts (see _run for the host-side shuffles)
    x_d = nc.dram_tensor("x", [128, NJ, D], F32, kind="ExternalInput").ap()
    adj_d = nc.dram_tensor("adj", [2, 128, NJ, D], I32, kind="ExternalInput").ap()
    ut_d = nc.dram_tensor("ut", [128, 4, D], F32, kind="ExternalInput").ap()
    out_d = nc.dram_tensor("out", [128, 8, D], F32, kind="ExternalOutput").ap()

    with tile.TileContext(nc) as tc:
        with (
            tc.tile_pool(name="xw", bufs=1) as xw_pool,
            tc.tile_pool(name="abf", bufs=8) as abf_pool,
            tc.tile_pool(name="cons", bufs=1) as cons_pool,
            tc.tile_pool(name="evac", bufs=2) as evac_pool,
            tc.tile_pool(name="osb", bufs=2) as osb_pool,
            tc.tile_pool(name="pacc", bufs=1, space="PSUM") as pacc_pool,
            tc.tile_pool(name="pout", bufs=1, space="PSUM") as pout_pool,
        ):
            ones = cons_pool.tile([128, 128], BF16)
            nc.vector.memset(ones[:], 1.0)
            u_bf = cons_pool.tile([128, 4, D], BF16)

            xg_tiles = [None] * NG

            def load_x_group(g):
                xg = xw_pool.tile([128, XG, D], BF16, tag=f"xg{g}", name=f"xg{g}")
                nc.gpsimd.dma_start(xg[:], x_d[:, g * XG:(g + 1) * XG, :])
                xg_tiles[g] = xg

            for h in range(2):
                agg_ps = [
                    pacc_pool.tile(
                        [128, D], F32, tag=f"agg{c}", name=f"agg{c}",
                        bufs=2 if c == 0 else 1,
                    )
                    for c in range(4)
                ]
                deg_ps = pacc_pool.tile([128, D], F32, tag="deg", bufs=2)
                for g in range(NG):
                    # interleave x-group loads with adj groups on the SWDGE
                    # queue so neither stream starves the other
                    if h == 0:
                        load_x_group(g)
                    a_bf = abf_pool.tile([128, XG, D], BF16, tag="abf")
                    nc.gpsimd.dma_start(
                        a_bf[:], adj_d[h, :, g * XG:(g + 1) * XG, :]
                    )
                    if h == 0 and g == 0:
                        nc.gpsimd.dma_start(u_bf[:], ut_d[:])
                    xg = xg_tiles[g]
                    for t in range(XG):
                        j = g * XG + t
                        st, sp = j == 0, j == NJ - 1
                        nc.tensor.matmul(
                            deg_ps[:], ones[:], a_bf[:, t, :], start=st, stop=sp
                        )
                        for c in range(4):
                            nc.tensor.matmul(
                                agg_ps[c][:],
                                xg[:, t, c * 128:(c + 1) * 128],
                                a_bf[:, t, :],
                                start=st,
                                stop=sp,
                            )

                recip = evac_pool.tile([128, D], F32, tag="recip")
                nc.vector.reciprocal_approx_fast(recip[:], deg_ps[:])
                agg_sc = [
                    evac_pool.tile([128, D], BF16, tag=f"aggsc{c}", name=f"aggsc{c}")
                    for c in range(4)
                ]
                for c in range(4):
                    nc.vector.tensor_mul(agg_sc[c][:], agg_ps[c][:], recip[:])

                out_sb = osb_pool.tile([128, 4, D], F32, tag="osb")
                for ic in range(4):
                    out_ps = pout_pool.tile([128, D], F32, tag="outps")
                    for c in range(4):
                        nc.tensor.matmul(
                            out_ps[:],
                            agg_sc[c][:, ic * 128:(ic + 1) * 128],
                            u_bf[:, c, :],
                            start=c == 0,
                            stop=c == 3,
                        )
                    nc.scalar.activation(
                        out_sb[:, ic, :], out_ps[:], mybir.ActivationFunctionType.Relu
                    )
                nc.sync.dma_start(out_d[:, h * 4:(h + 1) * 4, :], out_sb[:])

    nc.compile()
    return nc


def _get_compiled():
    global _compiled
    if _compiled is None:
        _compiled = _build()
    return _compiled


def _run(x, adj, u, **spmd_kwargs):
    nc = _get_compiled()
    x = np.asarray(x, dtype=np.float32)
    adj = np.asarray(adj, dtype=np.int32)
    u = np.asarray(u, dtype=np.float32)

    # x[t*128+p, d] -> x_r[p, t, d]
    x_r = np.ascontiguousarray(x.reshape(NJ, 128, D).transpose(1, 0, 2))
    # U^T[c*128+p, k] -> ut_r[p, c, k]
    ut_r = np.ascontiguousarray(u.T.reshape(4, 128, D).transpose(1, 0, 2))
    in_maps = []
    for core in range(NCORES):
        shard = adj[:, core * SH:(core + 1) * SH]
        # shard[t*128+p, h*512+d] -> adj_r[h, p, t, d]
        adj_r = np.ascontiguousarray(
            shard.reshape(NJ, 128, 2, D).transpose(2, 1, 0, 3)
        )
        in_maps.append({"x": x_r, "ut": ut_r, "adj": adj_r})

    res = run_bass_kernel_spmd(nc, in_maps, core_ids=list(range(NCORES)), **spmd_kwargs)
    # out_r[p, hic, k] -> out[hic*128+p, k], then stack core slabs
    out = np.concatenate(
        [
            res.results[c]["out"].transpose(1, 0, 2).reshape(SH, D)
            for c in range(NCORES)
        ],
        axis=0,
    )
    return out, res


def kernel(x, adj, U):
    out, _ = _run(x, adj, U)
    return out
